# revision 4
# baseline (speedup 1.0000x reference)
"""Biaffine labeler kernel for 8 Trainium2 NeuronCores.

Computation (full shapes):
    dep  [2, 2048, 1024], head [2, 2049, 1024], head_indices [2, 2048]
    dep_label  = dep @ dep_W.T + dep_b                    [2, 2048, 512]
    selected   = (head gathered at head_indices) @ head_W.T + head_b
    logits[b,t,n] = dep_label[b,t,:] @ W[n] @ selected[b,t,:] + bias[n]

Sharding: data-parallel over (b, t): core c handles b = c // 4 and the
512-token range starting at (c % 4) * 512.  W / projections replicated.

Per-core device program:
    1. dma_gather the 512 predicted-head rows (4KB each) from HBM (SWDGE)
    2. PE-transpose dep slice / gathered rows / dep_W / head_W to put the
       contraction dim on partitions; PSUM->SBUF copies (ACT) cast to bf16
    3. bf16 projections:  dep_labelT [512e, 512t],  selected [512t, 512e]
       (biases folded in as K=1 rank-1 matmuls into the same PSUM group)
    4. per label n: stream W[n] via SWDGE casting DMA (fp32 HBM -> bf16
       SBUF), A_n = dep_label @ W[n] on PE (4x4 K-tiles, N=512), then
       DVE multiply by selected and ACT accumulate over e per 128-token
       chunk; add bias at the end.
"""

import sys

for _p in ("/opt/trn_rl_repo", "/root/.axon_site/_ro/trn_rl_repo"):
    if _p not in sys.path:
        sys.path.append(_p)

from contextlib import ExitStack

import numpy as np

import concourse.bass as bass  # noqa: F401
import concourse.mybir as mybir
import concourse.tile as tile
from concourse import bacc
from concourse.bass_utils import run_bass_kernel_spmd
from concourse.tile_rust import add_dep_helper

B, T, D = 2, 2048, 1024
E = 512            # label-space dim (D // 2)
NLAB = 50
NCORES = 8
TLOC = (B * T) // NCORES   # 512 tokens per core
TP = TLOC // 128           # 4 token chunks
DP = D // 128              # 8 contraction chunks for the projections
EP = E // 128              # 4 chunks of the label dim
HEADT = T + 1

F32 = mybir.dt.float32
BF16 = mybir.dt.bfloat16
I16 = mybir.dt.int16


def _raw(inst):
    return getattr(inst, "ins", inst)


def build_program():
    nc = bacc.Bacc("TRN2", target_bir_lowering=False, debug=False,
                   num_devices=NCORES)

    dep_s = nc.dram_tensor("dep_s", [TLOC, D], F32, kind="ExternalInput").ap()
    headf = nc.dram_tensor("headf", [HEADT, D], F32, kind="ExternalInput").ap()
    idxs = nc.dram_tensor("idxs", [128, TLOC // 16], I16,
                          kind="ExternalInput").ap()
    depW = nc.dram_tensor("depW", [E, D], F32, kind="ExternalInput").ap()
    headW = nc.dram_tensor("headW", [E, D], F32, kind="ExternalInput").ap()
    depb = nc.dram_tensor("depb", [1, E], F32, kind="ExternalInput").ap()
    headb = nc.dram_tensor("headb", [1, E], F32, kind="ExternalInput").ap()
    Wbig = nc.dram_tensor("Wbig", [NLAB, E, E], F32, kind="ExternalInput").ap()
    biasn = nc.dram_tensor("biasn", [1, NLAB], F32, kind="ExternalInput").ap()
    identd = nc.dram_tensor("identd", [128, 128], F32,
                            kind="ExternalInput").ap()
    logits = nc.dram_tensor("logits", [TLOC, NLAB], F32,
                            kind="ExternalOutput").ap()

    with tile.TileContext(nc) as tc, ExitStack() as ctx:
        # ---- persistent tiles (one pool, one slot per distinct tag) ----
        pp = ctx.enter_context(tc.tile_pool(name="persist", bufs=1))

        def ptile(shape, dtype, name):
            return pp.tile(shape, dtype, tag=name, name=name)

        ident = ptile([128, 128], F32, "ident")
        ones_r = ptile([1, TLOC], BF16, "ones_r")
        stage_f32 = ptile([1, E], F32, "stage_f32")    # bias staging
        depb_sb = ptile([1, E], BF16, "depb_sb")
        headb_sb = ptile([1, E], BF16, "headb_sb")
        biasn_f32 = ptile([1, NLAB], F32, "biasn_f32")
        biasn_sb = ptile([1, NLAB], BF16, "biasn_sb")
        bias_bc = ptile([128, NLAB], F32, "bias_bc")
        idx_sb = ptile([128, TLOC // 16], I16, "idx_sb")
        dep_lT = ptile([128, EP, TLOC], BF16, "dep_lT")   # [e, tok]
        sel_sb = ptile([128, TP, E], BF16, "sel_sb")      # [tok, e]
        dep_sT = ptile([128, DP, TLOC], BF16, "dep_sT")   # [d, tok]
        sel_raw = ptile([128, TP, D], F32, "sel_raw")     # [tok, d]
        sel_rT = ptile([128, DP, TLOC], BF16, "sel_rT")   # [d, tok]
        depWT = ptile([128, DP, E], BF16, "depWT")        # [d, e]
        headWT = ptile([128, DP, E], BF16, "headWT")      # [d, e]
        logit_sb = ptile([128, TP, NLAB], F32, "logit_sb")
        logit_out = ptile([128, TP, NLAB], F32, "logit_out")

        ld_pool = ctx.enter_context(tc.tile_pool(name="ld", bufs=4))
        w_pool = ctx.enter_context(tc.tile_pool(name="wn", bufs=3))
        scr_pool = ctx.enter_context(tc.tile_pool(name="scr", bufs=3))
        dead_pool = ctx.enter_context(tc.tile_pool(name="dead", bufs=2))
        ps_pool = ctx.enter_context(
            tc.tile_pool(name="ps", bufs=6, space="PSUM"))

        nc.sync.dma_start(ident[:], identd)
        nc.vector.memset(ones_r[:], 1.0)
        nc.sync.dma_start(idx_sb[:], idxs)
        # bias vectors: fp32 load, ACT cast to bf16
        nc.sync.dma_start(stage_f32[:], depb)
        nc.scalar.copy(depb_sb[:], stage_f32[:])
        nc.sync.dma_start(stage_f32[:], headb)
        nc.scalar.copy(headb_sb[:], stage_f32[:])
        nc.sync.dma_start(biasn_f32[:], biasn)
        nc.scalar.copy(biasn_sb[:], biasn_f32[:])

        # bias[n] broadcast across partitions: ones[128]^T x biasn[50]
        psb = ps_pool.tile([128, 512], F32, tag="ps")
        nc.tensor.matmul(psb[:, :NLAB], ones_r[:, :128], biasn_sb[:],
                         start=True, stop=True)
        nc.scalar.copy(bias_bc[:], psb[:, :NLAB])

        # gather the predicted-head rows for this core's 512 tokens.
        # Must be the first SWDGE op: the mlp library load that precedes it
        # must not race in-flight SWDGE casting DMAs.
        gather_inst = nc.gpsimd.dma_gather(
            out_ap=sel_raw[:],
            in_ap=headf,
            idxs_ap=idx_sb[:],
            num_idxs=TLOC,
            num_idxs_reg=TLOC,
            elem_size=D,
        )

        def transpose_to(dst, srcs, nblk):
            # srcs(i, j) yields the [128, 128] block for free-chunk i /
            # d-chunk j; dst[:, j, :] collects nblk transposed blocks via
            # one PSUM bank; copy casts fp32 -> dst dtype on ACT.
            for j in range(DP):
                psj = ps_pool.tile([128, 512], F32, tag="ps")
                for i in range(nblk):
                    nc.tensor.transpose(psj[:, i * 128:(i + 1) * 128],
                                        srcs(i, j), ident[:])
                nc.scalar.copy(dst[:, j, :], psj[:, :nblk * 128])

        # dep slice: load natural [tok, d], transpose to [d, tok] bf16
        dep_tiles = []
        for i in range(TP):
            t = ld_pool.tile([128, D], F32, tag="ld")
            nc.sync.dma_start(t[:], dep_s[i * 128:(i + 1) * 128, :])
            dep_tiles.append(t)
        transpose_to(dep_sT,
                     lambda i, j: dep_tiles[i][:, j * 128:(j + 1) * 128], TP)

        # weights: load natural [e, d], transpose to [d, e] bf16
        for src_dram, dstT in ((depW, depWT), (headW, headWT)):
            wtiles = []
            for i in range(EP):
                t = ld_pool.tile([128, D], F32, tag="ld")
                nc.sync.dma_start(t[:], src_dram[i * 128:(i + 1) * 128, :])
                wtiles.append(t)
            transpose_to(
                dstT,
                lambda i, j, w=wtiles: w[i][:, j * 128:(j + 1) * 128], EP)

        # gathered head rows: [tok, d] -> [d, tok] bf16
        transpose_to(sel_rT,
                     lambda i, j: sel_raw[:, i, j * 128:(j + 1) * 128], TP)

        # dep projection -> dep_labelT [e, tok]; bias via K=1 rank-1 matmul
        for i in range(EP):
            psp = ps_pool.tile([128, 512], F32, tag="ps")
            for j in range(DP):
                nc.tensor.matmul(psp[:], depWT[:, j, i * 128:(i + 1) * 128],
                                 dep_sT[:, j, :], start=(j == 0), stop=False)
            nc.tensor.matmul(psp[:], depb_sb[:, i * 128:(i + 1) * 128],
                             ones_r[:], start=False, stop=True)
            nc.scalar.copy(dep_lT[:, i, :], psp[:])

        # head projection of gathered rows -> selected [tok, e]
        for i in range(TP):
            psp = ps_pool.tile([128, 512], F32, tag="ps")
            for j in range(DP):
                nc.tensor.matmul(psp[:], sel_rT[:, j, i * 128:(i + 1) * 128],
                                 headWT[:, j, :], start=(j == 0), stop=False)
            nc.tensor.matmul(psp[:], ones_r[:, :128], headb_sb[:],
                             start=False, stop=True)
            nc.scalar.copy(sel_sb[:, i, :], psp[:])

        # biaffine: per label, stream W[n] as bf16 (SWDGE casting DMA),
        # A_n = dep_label @ W[n] on PE, multiply by selected (DVE) and
        # accumulate over e (ACT accum_out)
        first_cast = None
        for n in range(NLAB):
            wt = w_pool.tile([128, EP, E], BF16, tag="wn")
            cast_dma = nc.gpsimd.dma_start(
                wt[:], Wbig[n].rearrange("(j p) e -> p j e", p=128))
            if first_cast is None:
                first_cast = cast_dma
                add_dep_helper(_raw(first_cast), _raw(gather_inst),
                               sync=True,
                               reason="SWDGE casts must follow dma_gather "
                                      "(mlp library load ordering)")
            for i in range(TP):
                psa = ps_pool.tile([128, 512], F32, tag="ps")
                for j in range(EP):
                    nc.tensor.matmul(psa[:],
                                     dep_lT[:, j, i * 128:(i + 1) * 128],
                                     wt[:, j, :],
                                     start=(j == 0), stop=(j == EP - 1))
                scr = scr_pool.tile([128, E], F32, tag="scr")
                nc.vector.tensor_tensor(out=scr[:], in0=psa[:],
                                        in1=sel_sb[:, i, :],
                                        op=mybir.AluOpType.mult)
                dead = dead_pool.tile([128, E], BF16, tag="dead")
                nc.scalar.activation(dead[:], scr[:],
                                     mybir.ActivationFunctionType.Copy,
                                     accum_out=logit_sb[:, i, n:n + 1])

        # add the label bias and store
        for i in range(TP):
            nc.vector.tensor_add(logit_out[:, i, :], logit_sb[:, i, :],
                                 bias_bc[:])
        nc.sync.dma_start(logits.rearrange("(i p) n -> p i n", p=128),
                          logit_out[:])

    nc.compile()
    return nc


_NC_CACHE = []


def _get_program():
    if not _NC_CACHE:
        _NC_CACHE.append(build_program())
    return _NC_CACHE[0]


def make_in_maps(dep, head, head_indices, dep_W, dep_b, head_W, head_b, W,
                 bias):
    dep = np.ascontiguousarray(dep, dtype=np.float32)
    head = np.ascontiguousarray(head, dtype=np.float32)
    shared = {
        "depW": np.ascontiguousarray(dep_W, dtype=np.float32),
        "headW": np.ascontiguousarray(head_W, dtype=np.float32),
        "depb": np.ascontiguousarray(dep_b, dtype=np.float32).reshape(1, E),
        "headb": np.ascontiguousarray(head_b, dtype=np.float32).reshape(1, E),
        "Wbig": np.ascontiguousarray(W, dtype=np.float32),
        "biasn": np.ascontiguousarray(bias, dtype=np.float32).reshape(1, NLAB),
        "identd": np.eye(128, dtype=np.float32),
    }
    in_maps = []
    cores_per_b = NCORES // B
    for c in range(NCORES):
        b = c // cores_per_b
        t0 = (c % cores_per_b) * TLOC
        idx = np.asarray(head_indices[b, t0:t0 + TLOC]).astype(np.int16)
        # dma_gather index layout: wrapped into 16 partitions
        # (i -> [i % 16, i // 16]), replicated over the 8 Q7 cores
        wrapped = np.ascontiguousarray(
            np.tile(idx.reshape(TLOC // 16, 16).T, (8, 1)))
        in_maps.append({
            "dep_s": np.ascontiguousarray(dep[b, t0:t0 + TLOC]),
            "headf": head[b],
            "idxs": wrapped,
            **shared,
        })
    return in_maps


def run_sharded(inputs, trace=False):
    """Run the SPMD kernel; returns (full_logits, BassKernelResults)."""
    nc = _get_program()
    in_maps = make_in_maps(
        inputs["dep"], inputs["head"], inputs["head_indices"],
        inputs["dep_W"], inputs["dep_b"], inputs["head_W"],
        inputs["head_b"], inputs["W"], inputs["bias"])
    res = run_bass_kernel_spmd(nc, in_maps, list(range(NCORES)), trace=trace)
    out = np.empty((B, T, NLAB), dtype=np.float32)
    cores_per_b = NCORES // B
    for c in range(NCORES):
        b = c // cores_per_b
        t0 = (c % cores_per_b) * TLOC
        out[b, t0:t0 + TLOC] = res.results[c]["logits"]
    return out, res


def kernel(dep, head, head_indices, mask, dep_W, dep_b, head_W, head_b, W,
           bias):
    out, _ = run_sharded({
        "dep": dep, "head": head, "head_indices": head_indices,
        "dep_W": dep_W, "dep_b": dep_b, "head_W": head_W,
        "head_b": head_b, "W": W, "bias": bias,
    })
    return out
